# revision 1
# baseline (speedup 1.0000x reference)
"""Trainium2 Bass kernel for windowed (Swin-style) multi-head attention.

Problem: nn_DiffeomorphicAttention  (B=16384 windows, N=49 tokens, C=128,
H=4 heads, hd=32, nW=64 shift masks, MLP relative-position bias).

Strategy: data-parallel over the window-batch axis across 8 NeuronCores
(2048 windows/core).  Per core, windows are processed in iterations of
G=8 windows (4 "pairs" of 2 windows fused into 98-token tiles; the
cross-window blocks of the 98x98 attention matrix are masked to -3e4 so
exp() zeroes them).

Per-pair dataflow (all fp32):
  x [98,128] --PE transpose--> xT [128,98]
  qT = Wq^T xT, kT = Wk^T xT        (PE, shared stationary Wq/Wk)
  v  = x Wv                          (PE, stationary xT)
  S^T = A^T (identity-matmul preload) + accumulated per-head K Q^T
        (PE row-tiled, tile_position=(32h,0))
  P = exp(S^T)                       (ACT, PSUM->SBUF)
  out_ext = P^T-stationary AV with ones-column => [out | rowsums]
  out = out_ext[:,:32] * 1/rowsums   (DVE, stride-0 broadcast)
  out --PE transpose--> outT; y = outT^T Wp  (PE)

The MLP bias table + masks are precomputed on the host into a combined
additive table A^T[32 pair-classes, 98, 4, 98] (masks repeat with period
64 windows = 32 pairs).  qkv scale is folded into Wq; k-bias is dropped
(softmax row-shift invariant); v-bias and proj bias are folded into a
host-side output bias.
"""

import numpy as np
from contextlib import ExitStack

WS = 7
N = 49
H = 4
DIM = 128
HD = 32
B_ = 16384
NW = 64
NCORES = 8
BWC = B_ // NCORES          # 2048 windows per core
G = 8                       # windows per iteration
PAIRS = G // 2
NITER = BWC // G            # 256
NEG = -30000.0

_cache = {}


def _host_bias_table(mlp_w1, mlp_b1, mlp_w2, mlp_b2, mask):
    """bias[h,n,m] from the coord MLP (exact-erf gelu), plus combined A^T."""
    import math
    ch = np.arange(WS, dtype=np.float64)
    hh, ww = np.meshgrid(ch, ch, indexing="ij")
    flat = np.stack([hh.ravel(), ww.ravel()])              # [2, N]
    rel = (flat[:, :, None] - flat[:, None, :]).transpose(1, 2, 0)  # [N,N,2]
    rel = rel / max(WS - 1, 1)
    z = rel @ mlp_w1.astype(np.float64) + mlp_b1.astype(np.float64)
    g = 0.5 * z * (1.0 + np.vectorize(math.erf)(z / math.sqrt(2.0)))
    bias = g @ mlp_w2.astype(np.float64) + mlp_b2.astype(np.float64)  # [N,N,H]
    bias = bias.transpose(2, 0, 1)                          # [H, n, m]
    # A[w,h,n,m] = bias + mask ; we need A^T[w,h,m,n]
    A = bias[None] + mask.astype(np.float64)[:, None]       # [64,4,49,49]
    AT = A.transpose(0, 1, 3, 2)                            # [64,4,m,n]
    # pair-class table: [32, 98(m'), 4, 98(n')]
    t = np.full((32, 98, H, 98), NEG, dtype=np.float64)
    for pc in range(32):
        t[pc, 0:49, :, 0:49] = AT[2 * pc].transpose(1, 0, 2)
        t[pc, 49:98, :, 49:98] = AT[2 * pc + 1].transpose(1, 0, 2)
    # device layout [98, 32, 4, 98]
    return np.ascontiguousarray(t.transpose(1, 0, 2, 3)).astype(np.float32)


def _build_program(niter=NITER):
    import concourse.bass as bass
    import concourse.tile as tile
    from concourse import bacc, mybir
    from concourse.masks import make_identity

    f32 = mybir.dt.float32
    nc = bacc.Bacc(None, target_bir_lowering=False)

    x_d = nc.dram_tensor("x", [niter * G, N, DIM], f32, kind="ExternalInput")
    r_d = nc.dram_tensor("r", [H, DIM, DIM], f32, kind="ExternalInput")
    wv_d = nc.dram_tensor("wv", [DIM, DIM], f32, kind="ExternalInput")
    wp_d = nc.dram_tensor("wp", [DIM, DIM], f32, kind="ExternalInput")
    a2_d = nc.dram_tensor("a2t", [98, 32 * H * 98], f32, kind="ExternalInput")
    y_d = nc.dram_tensor("y", [niter * G, N, DIM], f32, kind="ExternalOutput")

    WROW = N * DIM        # 6272 elements per window in DRAM

    with ExitStack() as ctx:
        tc = ctx.enter_context(tile.TileContext(nc))
        const = ctx.enter_context(tc.tile_pool(name="const", bufs=1))
        sbx = ctx.enter_context(tc.tile_pool(name="sbx", bufs=3))
        sbq = ctx.enter_context(tc.tile_pool(name="sbq", bufs=2))
        sbp = ctx.enter_context(tc.tile_pool(name="sbp", bufs=3))
        sbo = ctx.enter_context(tc.tile_pool(name="sbo", bufs=2))
        # PSUM pools — exactly 8 banks total
        ps_z = ctx.enter_context(tc.tile_pool(name="ps_z", bufs=1, space="PSUM"))
        ps_v = ctx.enter_context(tc.tile_pool(name="ps_v", bufs=1, space="PSUM"))
        ps_s = ctx.enter_context(tc.tile_pool(name="ps_s", bufs=1, space="PSUM"))
        ps_av = ctx.enter_context(tc.tile_pool(name="ps_av", bufs=2, space="PSUM"))
        ps_m = ctx.enter_context(tc.tile_pool(name="ps_m", bufs=2, space="PSUM"))

        # ---- constants ----
        r_s = const.tile([DIM, H, DIM], f32)
        wv_s = const.tile([DIM, DIM], f32)
        wp_s = const.tile([DIM, DIM], f32)
        a2_s = const.tile([98, 32, H, 98], f32)
        i98 = const.tile([98, 98], f32)
        dma = nc.default_dma_engine
        dma.dma_start(out=r_s,
                      in_=bass.AP(tensor=r_d, offset=0,
                                  ap=[[DIM, DIM], [DIM * DIM, H], [1, DIM]]))
        dma.dma_start(out=wv_s, in_=wv_d[:, :])
        dma.dma_start(out=wp_s, in_=wp_d[:, :])
        dma.dma_start(
            out=a2_s,
            in_=bass.AP(tensor=a2_d, offset=0,
                        ap=[[32 * H * 98, 98], [H * 98, 32], [98, H], [1, 98]]),
        )
        make_identity(nc, i98)

        for it in range(niter):
            b0 = it * G
            # ---- load X: partitions 0-48 even windows, 49-97 odd ----
            X = sbx.tile([98, PAIRS, DIM], f32)       # [98, 4, 128]
            in_even = bass.AP(tensor=x_d, offset=b0 * WROW,
                              ap=[[DIM, N], [2 * WROW, PAIRS], [1, DIM]])
            in_odd = bass.AP(tensor=x_d, offset=(b0 + 1) * WROW,
                             ap=[[DIM, N], [2 * WROW, PAIRS], [1, DIM]])
            dma.dma_start(out=X[0:N], in_=in_even)
            dma.dma_start(out=X[N:98], in_=in_odd)

            # ---- transpose X -> XT [128, 4*98] ----
            xt_ps = ps_m.tile([DIM, PAIRS * 98], f32, tag="m")
            for p in range(PAIRS):
                nc.tensor.transpose(xt_ps[:, p * 98:(p + 1) * 98], X[:, p, :], i98)
            XT = sbx.tile([DIM, PAIRS * 98], f32)
            nc.scalar.copy(XT, xt_ps)

            # ---- Z_h = R_h^T X^T  (per head, shared R stationary) ----
            Zsb = sbq.tile([DIM, H, PAIRS, 98], f32)
            for c in range(2):
                z_ps = ps_z.tile([DIM, H, 2, DIM], f32, tag="z")
                for h in range(H):
                    for j in range(2):
                        nc.tensor.matmul(
                            z_ps[:, h, j, 0:98], lhsT=r_s[:, h, :],
                            rhs=XT[:, (2 * c + j) * 98:(2 * c + j + 1) * 98],
                            start=True, stop=True)
                nc.scalar.copy(Zsb[:, :, 2 * c:2 * c + 2, :], z_ps[:, :, :, 0:98])

            # ---- V (natural) + ones column ----
            v_ps = ps_v.tile([98, PAIRS, H, HD], f32)
            for p in range(PAIRS):
                nc.tensor.matmul(v_ps[:, p], lhsT=XT[:, p * 98:(p + 1) * 98],
                                 rhs=wv_s, start=True, stop=True)
            Vsb = sbx.tile([98, PAIRS, H, HD + 1], f32)
            nc.gpsimd.memset(Vsb[:, :, :, HD:HD + 1], 1.0)
            nc.vector.tensor_copy(Vsb[:, :, :, 0:HD], v_ps)

            OUT = sbo.tile([98, PAIRS, H, HD], f32)
            for p in range(PAIRS):
                pc = (PAIRS * it + p) % 32
                # ---- S^T = A^T + sum_h K Q^T ----
                s_ps = ps_s.tile([98, H, 98], f32)
                nc.tensor.matmul(s_ps, lhsT=i98, rhs=a2_s[:, pc],
                                 start=True, stop=False)
                for h in range(H):
                    nc.tensor.matmul(
                        s_ps[:, h],
                        lhsT=XT[:, p * 98:(p + 1) * 98],
                        rhs=Zsb[:, h, p, :],
                        start=False, stop=(h == H - 1),
                    )
                # ---- P = exp(S^T) ----
                PT = sbp.tile([98, H, 98], f32)
                nc.scalar.activation(PT, s_ps, mybir.ActivationFunctionType.Exp)
                # ---- AV with ones column: [out | rowsum] ----
                av_ps = ps_av.tile([98, H, HD + 1], f32)
                for h in range(H):
                    nc.tensor.matmul(av_ps[:, h], lhsT=PT[:, h],
                                     rhs=Vsb[:, p, h], start=True, stop=True)
                rc = sbp.tile([98, H], f32)
                nc.vector.reciprocal(rc, av_ps[:, :, HD:HD + 1])
                for h in range(H):
                    nc.vector.tensor_scalar_mul(OUT[:, p, h], av_ps[:, h, 0:HD],
                                                rc[:, h:h + 1])

            # ---- out -> outT -> proj ----
            ox_ps = ps_m.tile([DIM, PAIRS * 98], f32, tag="m")
            for p in range(PAIRS):
                nc.tensor.transpose(ox_ps[:, p * 98:(p + 1) * 98],
                                    OUT[:, p, :, :], i98)
            OT = sbo.tile([DIM, PAIRS * 98], f32)
            nc.scalar.copy(OT, ox_ps)
            y_ps = ps_m.tile([98, PAIRS, DIM], f32, tag="m")
            for p in range(PAIRS):
                nc.tensor.matmul(y_ps[:, p], lhsT=OT[:, p * 98:(p + 1) * 98],
                                 rhs=wp_s, start=True, stop=True)
            Ysb = sbo.tile([98, PAIRS, DIM], f32)
            nc.vector.tensor_copy(Ysb, y_ps)

            out_even = bass.AP(tensor=y_d, offset=b0 * WROW,
                               ap=[[DIM, N], [2 * WROW, PAIRS], [1, DIM]])
            out_odd = bass.AP(tensor=y_d, offset=(b0 + 1) * WROW,
                              ap=[[DIM, N], [2 * WROW, PAIRS], [1, DIM]])
            dma.dma_start(out=out_even, in_=Ysb[0:N])
            dma.dma_start(out=out_odd, in_=Ysb[N:98])

    nc.compile()
    return nc


def kernel(x, mask, qkv_w, qkv_b, mlp_w1, mlp_b1, mlp_w2, mlp_b2,
           proj_w, proj_b):
    from concourse.bass_utils import run_bass_kernel_spmd

    x = np.asarray(x, dtype=np.float32)
    scale = HD ** (-0.5)
    wq = np.asarray(qkv_w[:, 0:DIM], np.float64) * scale
    wk = np.asarray(qkv_w[:, DIM:2 * DIM], np.float64)
    wv = np.ascontiguousarray(qkv_w[:, 2 * DIM:3 * DIM]).astype(np.float32)
    wp = np.ascontiguousarray(proj_w).astype(np.float32)
    # R_h = scale * Wq_h Wk_h^T  (S = X R X^T); q-bias must be zero here.
    assert np.abs(np.asarray(qkv_b[0:DIM])).max() == 0.0, "nonzero q-bias unsupported"
    R = np.stack([wq[:, 32 * h:32 * (h + 1)] @ wk[:, 32 * h:32 * (h + 1)].T
                  for h in range(H)]).astype(np.float32)   # [4,128,128]
    # k-bias: softmax-row-shift invariant -> dropped.
    # v-bias propagates through (rows of P sum to 1): y += bv @ Wp + bp (host).
    bv = np.asarray(qkv_b[2 * DIM:3 * DIM], dtype=np.float64)
    b_out = (bv @ np.asarray(proj_w, np.float64)
             + np.asarray(proj_b, np.float64)).astype(np.float32)
    a2t = _host_bias_table(np.asarray(mlp_w1), np.asarray(mlp_b1),
                           np.asarray(mlp_w2), np.asarray(mlp_b2),
                           np.asarray(mask)).reshape(98, 32 * H * 98)

    if "nc" not in _cache:
        _cache["nc"] = _build_program()
    nc = _cache["nc"]

    xs = x.reshape(NCORES, BWC, N, DIM)
    shared = {"r": R, "wv": wv, "wp": wp, "a2t": a2t}
    in_maps = [{"x": np.ascontiguousarray(xs[i]), **shared}
               for i in range(NCORES)]
    res = run_bass_kernel_spmd(nc, in_maps, core_ids=list(range(NCORES)))
    outs = [m["y"] for m in res.results]
    y = np.concatenate([o.reshape(BWC, N, DIM) for o in outs], axis=0)
    if np.abs(b_out).max() > 0:
        y = y + b_out[None, None, :]
    return y.astype(np.float32)



# revision 2
# speedup vs baseline: 888.1978x; 888.1978x over previous
"""Trainium2 Bass kernel for windowed (Swin-style) multi-head attention.

Problem: nn_DiffeomorphicAttention  (B=16384 windows, N=49 tokens, C=128,
H=4 heads, hd=32, nW=64 shift masks, MLP relative-position bias).

Strategy: data-parallel over the window-batch axis across 8 NeuronCores
(2048 windows/core).  Per core, windows are processed in iterations of
G=8 windows (4 "pairs" of 2 windows fused into 98-token tiles; the
cross-window blocks of the 98x98 attention matrix are masked to -3e4 so
exp() zeroes them).

Per-pair dataflow (all fp32):
  x [98,128] --PE transpose--> xT [128,98]
  qT = Wq^T xT, kT = Wk^T xT        (PE, shared stationary Wq/Wk)
  v  = x Wv                          (PE, stationary xT)
  S^T = A^T (identity-matmul preload) + accumulated per-head K Q^T
        (PE row-tiled, tile_position=(32h,0))
  P = exp(S^T)                       (ACT, PSUM->SBUF)
  out_ext = P^T-stationary AV with ones-column => [out | rowsums]
  out = out_ext[:,:32] * 1/rowsums   (DVE, stride-0 broadcast)
  out --PE transpose--> outT; y = outT^T Wp  (PE)

The MLP bias table + masks are precomputed on the host into a combined
additive table A^T[32 pair-classes, 98, 4, 98] (masks repeat with period
64 windows = 32 pairs).  qkv scale is folded into Wq; k-bias is dropped
(softmax row-shift invariant); v-bias and proj bias are folded into a
host-side output bias.
"""

import numpy as np
from contextlib import ExitStack

WS = 7
N = 49
H = 4
DIM = 128
HD = 32
B_ = 16384
NW = 64
NCORES = 8
BWC = B_ // NCORES          # 2048 windows per core
G = 8                       # windows per iteration
PAIRS = G // 2
NITER = BWC // G            # 256
NEG = -30000.0

_cache = {}


def _host_bias_table(mlp_w1, mlp_b1, mlp_w2, mlp_b2, mask):
    """bias[h,n,m] from the coord MLP (exact-erf gelu), plus combined A^T."""
    import math
    ch = np.arange(WS, dtype=np.float64)
    hh, ww = np.meshgrid(ch, ch, indexing="ij")
    flat = np.stack([hh.ravel(), ww.ravel()])              # [2, N]
    rel = (flat[:, :, None] - flat[:, None, :]).transpose(1, 2, 0)  # [N,N,2]
    rel = rel / max(WS - 1, 1)
    z = rel @ mlp_w1.astype(np.float64) + mlp_b1.astype(np.float64)
    g = 0.5 * z * (1.0 + np.vectorize(math.erf)(z / math.sqrt(2.0)))
    bias = g @ mlp_w2.astype(np.float64) + mlp_b2.astype(np.float64)  # [N,N,H]
    bias = bias.transpose(2, 0, 1)                          # [H, n, m]
    # A[w,h,n,m] = bias + mask ; we need A^T[w,h,m,n]
    A = bias[None] + mask.astype(np.float64)[:, None]       # [64,4,49,49]
    AT = A.transpose(0, 1, 3, 2)                            # [64,4,m,n]
    # pair-class table: [32, 98(m'), 4, 98(n')]
    t = np.full((32, 98, H, 98), NEG, dtype=np.float64)
    for pc in range(32):
        t[pc, 0:49, :, 0:49] = AT[2 * pc].transpose(1, 0, 2)
        t[pc, 49:98, :, 49:98] = AT[2 * pc + 1].transpose(1, 0, 2)
    # device layout [98, 32, 4, 98]
    return np.ascontiguousarray(t.transpose(1, 0, 2, 3)).astype(np.float32)


def _build_program(niter=NITER):
    import concourse.bass as bass
    import concourse.tile as tile
    from concourse import bacc, mybir
    from concourse.masks import make_identity

    f32 = mybir.dt.float32
    nc = bacc.Bacc(None, target_bir_lowering=False)

    x_d = nc.dram_tensor("x", [niter * G, N, DIM], f32, kind="ExternalInput")
    r_d = nc.dram_tensor("r", [H, DIM, DIM], f32, kind="ExternalInput")
    wv_d = nc.dram_tensor("wv", [DIM, DIM], f32, kind="ExternalInput")
    wp_d = nc.dram_tensor("wp", [DIM, DIM], f32, kind="ExternalInput")
    a2_d = nc.dram_tensor("a2t", [98, 32 * H * 98], f32, kind="ExternalInput")
    y_d = nc.dram_tensor("y", [niter * G, N, DIM], f32, kind="ExternalOutput")

    WROW = N * DIM        # 6272 elements per window in DRAM

    with ExitStack() as ctx:
        tc = ctx.enter_context(tile.TileContext(nc))
        const = ctx.enter_context(tc.tile_pool(name="const", bufs=1))
        sbx = ctx.enter_context(tc.tile_pool(name="sbx", bufs=3))
        sbq = ctx.enter_context(tc.tile_pool(name="sbq", bufs=2))
        sbp = ctx.enter_context(tc.tile_pool(name="sbp", bufs=3))
        sbo = ctx.enter_context(tc.tile_pool(name="sbo", bufs=2))
        # PSUM pools — exactly 8 banks total
        ps_z = ctx.enter_context(tc.tile_pool(name="ps_z", bufs=1, space="PSUM"))
        ps_v = ctx.enter_context(tc.tile_pool(name="ps_v", bufs=1, space="PSUM"))
        ps_s = ctx.enter_context(tc.tile_pool(name="ps_s", bufs=1, space="PSUM"))
        ps_av = ctx.enter_context(tc.tile_pool(name="ps_av", bufs=2, space="PSUM"))
        ps_m = ctx.enter_context(tc.tile_pool(name="ps_m", bufs=2, space="PSUM"))

        # ---- constants ----
        r_s = const.tile([DIM, H, DIM], f32)
        wv_s = const.tile([DIM, DIM], f32)
        wp_s = const.tile([DIM, DIM], f32)
        a2_s = const.tile([98, 32, H, 98], f32)
        i98 = const.tile([98, 98], f32)
        dma = nc.default_dma_engine
        dma.dma_start(out=r_s,
                      in_=bass.AP(tensor=r_d, offset=0,
                                  ap=[[DIM, DIM], [DIM * DIM, H], [1, DIM]]))
        dma.dma_start(out=wv_s, in_=wv_d[:, :])
        dma.dma_start(out=wp_s, in_=wp_d[:, :])
        dma.dma_start(
            out=a2_s,
            in_=bass.AP(tensor=a2_d, offset=0,
                        ap=[[32 * H * 98, 98], [H * 98, 32], [98, H], [1, 98]]),
        )
        make_identity(nc, i98)

        for it in range(niter):
            b0 = it * G
            # ---- load X: partitions 0-48 even windows, 49-97 odd ----
            X = sbx.tile([98, PAIRS, DIM], f32)       # [98, 4, 128]
            in_even = bass.AP(tensor=x_d, offset=b0 * WROW,
                              ap=[[DIM, N], [2 * WROW, PAIRS], [1, DIM]])
            in_odd = bass.AP(tensor=x_d, offset=(b0 + 1) * WROW,
                             ap=[[DIM, N], [2 * WROW, PAIRS], [1, DIM]])
            dma.dma_start(out=X[0:N], in_=in_even)
            dma.dma_start(out=X[N:98], in_=in_odd)

            # ---- transpose X -> XT [128, 4*98] ----
            xt_ps = ps_m.tile([DIM, PAIRS * 98], f32, tag="m")
            for p in range(PAIRS):
                nc.tensor.transpose(xt_ps[:, p * 98:(p + 1) * 98], X[:, p, :], i98)
            XT = sbx.tile([DIM, PAIRS * 98], f32)
            nc.scalar.copy(XT, xt_ps)

            # ---- Z_h = R_h^T X^T  (per head, shared R stationary) ----
            Zsb = sbq.tile([DIM, H, PAIRS, 98], f32)
            for c in range(2):
                z_ps = ps_z.tile([DIM, H, 2, DIM], f32, tag="z")
                for h in range(H):
                    for j in range(2):
                        nc.tensor.matmul(
                            z_ps[:, h, j, 0:98], lhsT=r_s[:, h, :],
                            rhs=XT[:, (2 * c + j) * 98:(2 * c + j + 1) * 98],
                            start=True, stop=True)
                nc.scalar.copy(Zsb[:, :, 2 * c:2 * c + 2, :], z_ps[:, :, :, 0:98])

            # ---- V (natural) + ones column ----
            v_ps = ps_v.tile([98, PAIRS, H, HD], f32)
            for p in range(PAIRS):
                nc.tensor.matmul(v_ps[:, p], lhsT=XT[:, p * 98:(p + 1) * 98],
                                 rhs=wv_s, start=True, stop=True)
            Vsb = sbx.tile([98, PAIRS, H, HD + 1], f32)
            nc.gpsimd.memset(Vsb[:, :, :, HD:HD + 1], 1.0)
            nc.vector.tensor_copy(Vsb[:, :, :, 0:HD], v_ps)

            OUT = sbo.tile([98, PAIRS, H, HD], f32)
            for p in range(PAIRS):
                pc = (PAIRS * it + p) % 32
                # ---- S^T = A^T + sum_h K Q^T ----
                s_ps = ps_s.tile([98, H, 98], f32)
                nc.tensor.matmul(s_ps, lhsT=i98, rhs=a2_s[:, pc],
                                 start=True, stop=False)
                for h in range(H):
                    nc.tensor.matmul(
                        s_ps[:, h],
                        lhsT=XT[:, p * 98:(p + 1) * 98],
                        rhs=Zsb[:, h, p, :],
                        start=False, stop=(h == H - 1),
                    )
                # ---- P = exp(S^T) ----
                PT = sbp.tile([98, H, 98], f32)
                nc.scalar.activation(PT, s_ps, mybir.ActivationFunctionType.Exp)
                # ---- AV with ones column: [out | rowsum] ----
                av_ps = ps_av.tile([98, H, HD + 1], f32)
                for h in range(H):
                    nc.tensor.matmul(av_ps[:, h], lhsT=PT[:, h],
                                     rhs=Vsb[:, p, h], start=True, stop=True)
                rc = sbp.tile([98, H], f32)
                nc.vector.reciprocal(rc, av_ps[:, :, HD:HD + 1])
                for h in range(H):
                    nc.vector.tensor_scalar_mul(OUT[:, p, h], av_ps[:, h, 0:HD],
                                                rc[:, h:h + 1])

            # ---- out -> outT -> proj ----
            ox_ps = ps_m.tile([DIM, PAIRS * 98], f32, tag="m")
            for p in range(PAIRS):
                nc.tensor.transpose(ox_ps[:, p * 98:(p + 1) * 98],
                                    OUT[:, p, :, :], i98)
            OT = sbo.tile([DIM, PAIRS * 98], f32)
            nc.scalar.copy(OT, ox_ps)
            y_ps = ps_m.tile([98, PAIRS, DIM], f32, tag="m")
            for p in range(PAIRS):
                nc.tensor.matmul(y_ps[:, p], lhsT=OT[:, p * 98:(p + 1) * 98],
                                 rhs=wp_s, start=True, stop=True)
            Ysb = sbo.tile([98, PAIRS, DIM], f32)
            nc.vector.tensor_copy(Ysb, y_ps)

            out_even = bass.AP(tensor=y_d, offset=b0 * WROW,
                               ap=[[DIM, N], [2 * WROW, PAIRS], [1, DIM]])
            out_odd = bass.AP(tensor=y_d, offset=(b0 + 1) * WROW,
                              ap=[[DIM, N], [2 * WROW, PAIRS], [1, DIM]])
            dma.dma_start(out=out_even, in_=Ysb[0:N])
            dma.dma_start(out=out_odd, in_=Ysb[N:98])

    nc.compile()
    return nc


def get_program():
    if "nc" not in _cache:
        _cache["nc"] = _build_program()
    return _cache["nc"]


def prepare_in_maps(inputs):
    """Host-side prep: returns (in_maps, postprocess(results)->y)."""
    x = np.asarray(inputs["x"], dtype=np.float32)
    qkv_w = np.asarray(inputs["qkv_w"])
    qkv_b = np.asarray(inputs["qkv_b"])
    proj_w = np.asarray(inputs["proj_w"])
    proj_b = np.asarray(inputs["proj_b"])
    scale = HD ** (-0.5)
    wq = np.asarray(qkv_w[:, 0:DIM], np.float64) * scale
    wk = np.asarray(qkv_w[:, DIM:2 * DIM], np.float64)
    wv = np.ascontiguousarray(qkv_w[:, 2 * DIM:3 * DIM]).astype(np.float32)
    wp = np.ascontiguousarray(proj_w).astype(np.float32)
    # R_h = scale * Wq_h Wk_h^T  (S = X R X^T); q-bias must be zero here.
    assert np.abs(np.asarray(qkv_b[0:DIM])).max() == 0.0, "nonzero q-bias unsupported"
    R = np.stack([wq[:, 32 * h:32 * (h + 1)] @ wk[:, 32 * h:32 * (h + 1)].T
                  for h in range(H)]).astype(np.float32)   # [4,128,128]
    # k-bias: softmax-row-shift invariant -> dropped.
    # v-bias propagates through (rows of P sum to 1): y += bv @ Wp + bp (host).
    bv = np.asarray(qkv_b[2 * DIM:3 * DIM], dtype=np.float64)
    b_out = (bv @ np.asarray(proj_w, np.float64)
             + np.asarray(proj_b, np.float64)).astype(np.float32)
    a2t = _host_bias_table(np.asarray(inputs["mlp_w1"]), np.asarray(inputs["mlp_b1"]),
                           np.asarray(inputs["mlp_w2"]), np.asarray(inputs["mlp_b2"]),
                           np.asarray(inputs["mask"])).reshape(98, 32 * H * 98)

    xs = x.reshape(NCORES, BWC, N, DIM)
    shared = {"r": R, "wv": wv, "wp": wp, "a2t": a2t}
    in_maps = [{"x": np.ascontiguousarray(xs[i]), **shared}
               for i in range(NCORES)]

    def postprocess(results):
        outs = [m["y"] for m in results]
        y = np.concatenate([o.reshape(BWC, N, DIM) for o in outs], axis=0)
        if np.abs(b_out).max() > 0:
            y = y + b_out[None, None, :]
        return y.astype(np.float32)

    return in_maps, postprocess


def kernel(x, mask, qkv_w, qkv_b, mlp_w1, mlp_b1, mlp_w2, mlp_b2,
           proj_w, proj_b):
    from concourse.bass_utils import run_bass_kernel_spmd

    in_maps, postprocess = prepare_in_maps(dict(
        x=x, mask=mask, qkv_w=qkv_w, qkv_b=qkv_b, mlp_w1=mlp_w1,
        mlp_b1=mlp_b1, mlp_w2=mlp_w2, mlp_b2=mlp_b2, proj_w=proj_w,
        proj_b=proj_b))
    nc = get_program()
    res = run_bass_kernel_spmd(nc, in_maps, core_ids=list(range(NCORES)))
    return postprocess(res.results)

